# revision 104
# baseline (speedup 1.0000x reference)
"""MoE routing mixture kernel for Trainium2 (8 NeuronCores, SPMD).

Math: out[b] = sum_k selection_score[b, idx[b,k]] * all_weight[idx[b,k]]
Rewritten as a dense matmul: out = C @ W_flat, where
  C[b,e]    = selection_score[b,e] * |{k : idx[b,k]==e}|      ([2048, 64])
  W_flat    = all_weight.reshape(64, 16384)

Sharding: 8 cores = 2 row-groups x 4 col-groups. Each core produces a
[1024, 4096] tile of the [2048, 16384] output. The big store is fp16
(the problem is DMA-roofline bound: fp32 out would be 16.8 MB/core,
fp16 is 8.4 MB; W slice per core is [64, 4096] fp16 = 0.5 MB).

Engine constraints that shape the design: DMA cannot touch PSUM, and
GPSIMD (Pool) cannot access PSUM either, so every matmul result must
pass through an ACT or DVE PSUM->SBUF copy — those copies are the
second-tightest resource after the DMA. Pool therefore handles the
SBUF-only one-hot work.

Per-core pipeline (raw Bass):
  SP  : loads in dependency order (mini: idx bits; rest: scoresT +
        W slice 0; W slices 1-3; W slices 4-7), then 33 output stores.
  Pool: generates iota + transpose identity on-chip at t~0, then per
        128-row chunk r>=1: 8x tensor_scalar is_equal + add tree
        -> cnt[r] (all SBUF; GPSIMD cannot touch PSUM).
  DVE : chunk 0's eq/reduce (fill path), then per chunk: ct[r] =
        ctp * scoresT (PSUM->SBUF move of C^T fused with the score
        multiply) + 3-4 of the 8 PSUM cast-copy slices.
  PE  : ~28 dummy transposes from t~0 (clock-ramp warm-up), then per
        chunk: transpose cnt[r] -> ctp (fp16 PSUM, hoisted between the
        previous chunk's matmuls); 8 fp16 matmuls [64,128]^T @ [64,512]
        -> fp32 PSUM, rotating over a 7-bank ring (bank = matmul % 7).
  ACT : 4-5 of the 8 cast-copy slices per chunk.

Output assembled on host: fp16 tiles -> fp32 [2048, 32, 512].
"""

import sys
from contextlib import ExitStack

import numpy as np

sys.path.insert(0, "/opt/trn_rl_repo")

BS, E, TOPK, PL, D = 2048, 64, 8, 32, 512
NF = PL * D  # 16384 flattened prompt*dim
N_CORES = 8
RG, CG = 2, 4  # row groups x col groups
ROWS = BS // RG  # 1024 rows per core
COLS = NF // CG  # 4096 cols per core
RCH = ROWS // 128  # 8 row chunks
NSL = COLS // D  # 8 matmul slices of 512 cols per chunk
NPS = 7  # psum ring banks

# packed small-input layout (fp16 cols), loaded as two DMAs:
#   mini [0 : 128)    = [128 idx-as-f32-bits]   (unblocks the eq chains)
#   rest [128 : 1152) = [512 scoT2 | 512 W slice 0 (partitions 0:64)]
# (iota and the transpose identity are generated on-chip by Pool)
PK_IDX = 0
PK_SCO = 2 * RCH * TOPK
PK_W0 = PK_SCO + 512
PKW = PK_W0 + D
PK_MINI = PK_SCO  # boundary between the two loads


def _group_units(g):
    """Copy units (tuples of slice indices) for group g, split by engine.

    The PSUM bank of matmul m is m % 7, so bank adjacency inside a group
    rotates with g; pairs are chosen so both slices sit in adjacent banks
    (single contiguous copy). ACT gets 5 slices (2 pairs + slice 7), DVE
    3 (1 pair + 1 single). The DVE single goes first when its ring
    deadline is tight (low slice index).
    """
    if g == 0:
        # fill path: all singles early; DVE does slice 0 inline right
        # after mul(0) while otherwise idle, so the first store (a [512]
        # store of slice 0) can issue as soon as possible
        return [(1,), (4, 5), (7,)], [(0,), (2, 3), (6,)]
    G = g % 7
    if G == 2:
        pairs, singles = [(0, 1), (2, 3), (5, 6)], [4, 7]
    elif G == 4:
        pairs, singles = [(0, 1), (3, 4), (5, 6)], [2, 7]
    elif G == 6:
        pairs, singles = [(1, 2), (3, 4), (5, 6)], [0, 7]
    else:
        pairs, singles = [(0, 1), (2, 3), (4, 5)], [6, 7]
    act = [pairs[0], pairs[2]]
    if singles[0] < pairs[1][0]:
        dve = [(singles[0],), pairs[1]]
    else:
        dve = [pairs[1], (singles[0],)]
    # slice 7 alternates between ACT and DVE to balance the two PSUM-copy
    # engines (~2.4us/chunk each; the 2.91us DMA store rate then paces)
    if g % 2 == 0:
        act.append((singles[1],))
    else:
        dve.append((singles[1],))
    return act, dve


def _copy_tables():
    """slice index (0..63) -> (engine, count) + per-group emission lists."""
    table = {}
    counts = {"A": 0, "D": 0}
    act_em, dve_em = [], []
    for g in range(RCH):
        act, dve = _group_units(g)
        act_em.append(act)
        dve_em.append(dve)
        for unit in act:
            counts["A"] += 1
            for s in unit:
                table[NSL * g + s] = ("A", counts["A"])
        for unit in dve:
            counts["D"] += 1
            for s in unit:
                table[NSL * g + s] = ("D", counts["D"])
    return table, act_em, dve_em


_SLICE_SEM, _ACT_EM, _DVE_EM = _copy_tables()

_cache: dict = {}


def _done_waits(*slices):
    need = {}
    for s in slices:
        sem, n = _SLICE_SEM[s]
        need[sem] = max(need.get(sem, 0), n)
    return sorted(need.items())


def _build_program():
    import concourse.bass as bass
    import concourse.mybir as mybir

    f16 = mybir.dt.float16
    f32 = mybir.dt.float32
    eq_op = mybir.AluOpType.is_equal
    add_op = mybir.AluOpType.add
    nc = bass.Bass()

    pk_d = nc.declare_dram_parameter("pk", [128, PKW], f16, isOutput=False)
    w_d = nc.declare_dram_parameter("wk", [E, COLS], f16, isOutput=False)
    out_d = nc.declare_dram_parameter("out", [ROWS, COLS], f16, isOutput=True)

    ctx = ExitStack()
    with ctx:
        sb = lambda tag, shape, dt=f16: ctx.enter_context(  # noqa: E731
            nc.sbuf_tensor(tag, shape, dt)
        )
        pk_t = sb("pk_t", [128, PKW])
        w_t = sb("w_t", [E, COLS])
        wu_t = sb("wu_t", [128, 128])  # never written: warm-up operand
        eq = [sb(f"eq{r}", [128, TOPK * E]) for r in range(RCH)]
        eq0 = eq[0]
        tr1 = sb("tr1", [128, 4 * E])
        tr2 = [sb(f"tr2_{i}", [128, 2 * E]) for i in range(2)]
        cnt = [sb(f"cnt{r}", [128, E]) for r in range(RCH)]
        ct = [sb(f"ct{r}", [E, 128]) for r in range(RCH)]
        stg = [sb(f"stg{r}", [128, COLS]) for r in range(RCH)]
        # on-chip constants (Pool generates these at t~0)
        i16 = mybir.dt.int16
        it_pm = sb("it_pm", [128, 128], i16)
        it_e = sb("it_e", [128, E], i16)
        iota_t = sb("iota_t", [128, E])
        ident_t = sb("ident_t", [128, 128])

        iota_ap = iota_t[:]
        ident_ap = ident_t[:]

        def idx_scalar(r, k):
            c = PK_IDX + 2 * (r * TOPK + k)
            return pk_t[:, c : c + 2].bitcast(f32)

        def scoT_ap(r):
            # scoT2[p, c]: p<64 -> scores.T[p, c]; p>=64 -> scores.T[p-64, 512+c]
            pbase = (r // 4) * E
            cbase = PK_SCO + (r % 4) * 128
            return pk_t[pbase : pbase + E, cbase : cbase + 128]

        ctp = ctx.enter_context(nc.psum_tensor("ctp", [E, 128], f16))
        pmall = ctx.enter_context(nc.psum_tensor("pmall", [128, NPS * D], f32))

        def pm_ap(bank, nbanks=1):
            return pmall[:, bank * D : (bank + nbanks) * D]

        s_in = ctx.enter_context(nc.semaphore("s_in"))
        s_mini = ctx.enter_context(nc.semaphore("s_mini"))
        s_w = ctx.enter_context(nc.semaphore("s_w"))
        s_const = ctx.enter_context(nc.semaphore("s_const"))
        s_eq0 = ctx.enter_context(nc.semaphore("s_eq0"))
        s_cnt = ctx.enter_context(nc.semaphore("s_cnt"))
        s_p2 = ctx.enter_context(nc.semaphore("s_p2"))
        s_tp = ctx.enter_context(nc.semaphore("s_tp"))
        s_c = ctx.enter_context(nc.semaphore("s_c"))
        s_mm = ctx.enter_context(nc.semaphore("s_mm"))
        s_cp = {
            "A": ctx.enter_context(nc.semaphore("s_cpa")),
            "D": ctx.enter_context(nc.semaphore("s_cpd")),
        }
        s_out = ctx.enter_context(nc.semaphore("s_out"))

        def emit_unit_copy(engobj, copy_fn, sem, g, unit):
            """PSUM->SBUF fp32->fp16 cast copy of `unit` (adjacent banks).
            The s_mm wait rides on the copy instruction itself (one wait +
            one update per descriptor), saving a standalone-wait SEQ slot."""
            j0, jn = unit[0], unit[-1]
            m_last = NSL * g + jn
            bank = (NSL * g + j0) % NPS
            ins = copy_fn(stg[g][:, j0 * D : (jn + 1) * D], pm_ap(bank, len(unit)))
            ins.wait_op(s_mm, m_last + 1, "sem-ge")
            ins.then_inc(sem, 1)

        block = ctx.enter_context(nc.Block())

        @block.sync
        def _(sp):
            sp.dma_start(out=pk_t[:, :PK_MINI], in_=pk_d[:, :PK_MINI]).then_inc(
                s_mini, 16
            )
            sp.dma_start(out=pk_t[:, PK_MINI:], in_=pk_d[:, PK_MINI:]).then_inc(
                s_in, 16
            )
            # W slice 0 rides in the rest load; slices 1-3 and 4-7 stream in
            sp.dma_start(out=w_t[:, D : COLS // 2], in_=w_d[:, D : COLS // 2]).then_inc(
                s_w, 16
            )
            sp.dma_start(out=w_t[:, COLS // 2 :], in_=w_d[:, COLS // 2 :]).then_inc(
                s_w, 16
            )
            # group 0 ships as 2x[512] + 3x[1024]; groups 1..7 as 4x[1024]
            stores = [(0, (0, 0)), (0, (1, 1)), (0, (2, 3)), (0, (4, 5)), (0, (6, 7))]
            stores += [
                (r, (2 * q, 2 * q + 1)) for r in range(1, RCH) for q in range(4)
            ]
            for r, (j0, j1) in stores:
                waits = _done_waits(NSL * r + j0, NSL * r + j1)
                # attach one wait to the DMA itself: its 565ns SEQ setup then
                # runs before the wait instead of after, so the transfer
                # launches sooner once the data lands (extras go standalone)
                for eng, n in waits[:-1]:
                    sp.wait_ge(s_cp[eng], n)
                rows = slice(r * 128, (r + 1) * 128)
                cols = slice(j0 * D, (j1 + 1) * D)
                ins = sp.dma_start(out=out_d[rows, cols], in_=stg[r][:, cols])
                eng, n = waits[-1]
                ins.wait_op(s_cp[eng], n, "sem-ge")
                ins.then_inc(s_out, 16)
            sp.wait_ge(s_out, 16 * len(stores))

        @block.gpsimd
        def _(gp):
            # on-chip constants, ready by t~1.5us (before any load completes):
            # iota row 0..63 and the 128x128 transpose identity
            gp.iota(it_e[:], [[1, E]], base=0, channel_multiplier=0)
            gp.tensor_copy(iota_t[:], it_e[:]).then_inc(s_const, 1)
            # it_pm[p, j] = j - p; identity = (it_pm == 0)
            gp.iota(it_pm[:], [[1, 128]], base=0, channel_multiplier=-1)
            gp.tensor_scalar(
                ident_t[:], it_pm[:], 0.0, None, eq_op
            ).then_inc(s_const, 1)
            # one-hot counts for chunks 1..7 (chunk 0 runs on DVE for a
            # faster pipeline fill); all operands SBUF-only. NOTE: Pool's
            # legal op set on V3 is narrow — tensor_tensor with broadcast
            # APs and scalar_tensor_tensor both fail the engine check, so
            # this stays 8x tensor_scalar + an add tree.
            for r in range(1, RCH):
                for k in range(TOPK):
                    ins = gp.tensor_scalar(
                        eq[r][:, k * E : (k + 1) * E],
                        iota_ap,
                        idx_scalar(r, k),
                        None,
                        eq_op,
                    )
                    if r == 1 and k == 0:
                        ins.wait_op(s_mini, 16, "sem-ge")
                gp.drain()
                gp.tensor_tensor(
                    tr1[:], eq[r][:, : 4 * E], eq[r][:, 4 * E :], add_op
                )
                gp.drain()
                ins = gp.tensor_tensor(
                    tr2[r % 2][:], tr1[:, : 2 * E], tr1[:, 2 * E :], add_op
                )
                if r >= 3:
                    # tr2 parity reuse: DVE's add3 of chunk r-2 must be done
                    ins.wait_op(s_cnt, r - 2, "sem-ge")
                ins.then_inc(s_p2, 1)
                gp.drain()

        @block.vector
        def _(v):
            v.wait_ge(s_const, 1)  # iota generated by Pool
            # chunk 0 count path on DVE (idle during fill). NOTE: summing the
            # one-hots via accumulating fp16 transposes in PSUM works in the
            # simulator but produces wrong results on hardware — keep this
            # eq + tensor_reduce + single transpose formulation. The mini-load
            # wait rides on the first eq.
            for k in range(TOPK):
                ins = v.tensor_scalar(
                    eq0[:, k * E : (k + 1) * E],
                    iota_ap,
                    idx_scalar(0, k),
                    None,
                    eq_op,
                )
                if k == 0:
                    ins.wait_op(s_mini, 16, "sem-ge")
            v.drain()
            # counts are small integers (<= 8): exact in fp16
            with nc.allow_low_precision(reason="counts <= 8 are exact in fp16"):
                v.tensor_reduce(
                    cnt[0][:],
                    eq0[:].rearrange("p (k e) -> p e k", k=TOPK),
                    mybir.AxisListType.X,
                    add_op,
                ).then_inc(s_eq0, 1)
            v.wait_ge(s_in, 16)  # scoT arrives with the rest load
            for r in range(RCH):
                if r == 1:
                    # chunk 0's pair copy before mul(1): its matmuls are done
                    # while mul(1) still waits on the hoisted transpose
                    emit_unit_copy(v, v.tensor_copy, s_cp["D"], 0, _DVE_EM[0][1])
                ins = v.tensor_mul(ct[r][:], ctp[:], scoT_ap(r))
                ins.wait_op(s_tp, r + 1, "sem-ge")
                ins.then_inc(s_c, 1)
                if r + 1 < RCH:
                    # final count add for chunk r+1 (Pool stops at its add2)
                    c = r + 1
                    ins = v.tensor_tensor(
                        cnt[c][:], tr2[c % 2][:, :E], tr2[c % 2][:, E:], add_op
                    )
                    ins.wait_op(s_p2, c, "sem-ge")
                    ins.then_inc(s_cnt, 1)
                if r == 0:
                    # slice 0 copy inline: DVE idle during fill
                    emit_unit_copy(v, v.tensor_copy, s_cp["D"], 0, _DVE_EM[0][0])
                else:
                    units = _DVE_EM[r - 1]
                    for unit in units[2:] if r == 1 else units:
                        emit_unit_copy(v, v.tensor_copy, s_cp["D"], r - 1, unit)
            for unit in _DVE_EM[RCH - 1]:
                emit_unit_copy(v, v.tensor_copy, s_cp["D"], RCH - 1, unit)

        @block.tensor
        def _(t):
            def tp(r):
                # ctp's previous reader (mul of r-1) is already done when
                # this runs: G(r-1) started, which required s_c >= r
                ins = t.transpose(ctp[:], cnt[r][:], ident_ap)
                ins.wait_op(s_cnt, r, "sem-ge")
                ins.then_inc(s_tp, 1)

            # pstate warm-up: the PE clock ramps to full only after ~3us of
            # near-continuous execution, and the ramp clock resets on long
            # idle gaps. Keep PE busy with dummy transposes (operand values
            # irrelevant; ctp is overwritten below) from t~0 until the
            # first real matmul so the whole fill phase runs at full clock.
            for _ in range(28):
                t.transpose(ctp[:], wu_t[:, 0:E], wu_t[:])
            t.wait_ge(s_const, 2)  # ident generated by Pool
            ins = t.transpose(ctp[:], cnt[0][:], ident_ap)
            ins.wait_op(s_eq0, 1, "sem-ge")  # cnt[0] from DVE
            ins.then_inc(s_tp, 1)
            for r in range(RCH):
                for j in range(NSL):
                    if r == 0 and j == 1:
                        t.wait_ge(s_w, 16)  # W slices 1-3
                    if r == 0 and j == 4:
                        t.wait_ge(s_w, 32)  # W slices 4-7
                    # hoist next chunk's transpose between matmuls, at the
                    # slot matching when Pool's count arrives (Pool lags
                    # early on and catches up by ~chunk 4) so PE never
                    # stalls mid-group before its ring-critical matmuls
                    if j == {0: 5, 1: 5, 2: 5, 3: 4}.get(r, 2) and r + 1 < RCH:
                        tp(r + 1)
                    m = r * NSL + j
                    ring = _done_waits(m - NPS) if m >= NPS else []
                    for eng, n in ring[:-1]:
                        t.wait_ge(s_cp[eng], n)
                    # W slice 0 lives in the pk tensor (rides the rest load)
                    rhs = (
                        pk_t[:E, PK_W0:] if j == 0 else w_t[:, j * D : (j + 1) * D]
                    )
                    if j == 0 and ring:
                        # first matmul carries the ct-ready wait; its ring
                        # wait (bank of m-7) then goes standalone
                        eng, n = ring.pop()
                        t.wait_ge(s_cp[eng], n)
                    ins = t.matmul(
                        pm_ap(m % NPS),
                        ct[r][:],
                        rhs,
                        start=True,
                        stop=True,
                    )
                    if j == 0:
                        ins.wait_op(s_c, r + 1, "sem-ge")  # ct[r] ready
                    elif ring:
                        # bank m%7 was last written by matmul m-7
                        eng, n = ring[-1]
                        ins.wait_op(s_cp[eng], n, "sem-ge")
                    ins.then_inc(s_mm, 1)

        @block.scalar
        def _(a):
            for r in range(RCH):
                for unit in _ACT_EM[r]:
                    emit_unit_copy(a, a.copy, s_cp["A"], r, unit)

    return nc


def _prep_inputs(selection_score, expert_indices, all_weight):
    scores = np.asarray(selection_score, dtype=np.float32)
    idx = np.asarray(expert_indices)
    w = np.asarray(all_weight, dtype=np.float32).reshape(E, NF).astype(np.float16)

    in_maps = []
    for core in range(N_CORES):
        rg, cg = divmod(core, CG)
        rsl = slice(rg * ROWS, (rg + 1) * ROWS)
        scoT = scores[rsl].T.astype(np.float16)  # [64, 1024]
        idxp = np.ascontiguousarray(
            idx[rsl]
            .astype(np.float32)
            .reshape(RCH, 128, TOPK)
            .transpose(1, 0, 2)
            .reshape(128, RCH * TOPK)
        )
        wk = np.ascontiguousarray(w[:, cg * COLS : (cg + 1) * COLS])
        pk = np.zeros((128, PKW), dtype=np.float16)
        pk[:, PK_IDX:PK_SCO] = idxp.view(np.float16)
        pk[:E, PK_SCO:PK_W0] = scoT[:, :512]
        pk[E:, PK_SCO:PK_W0] = scoT[:, 512:]
        pk[:E, PK_W0:] = wk[:, :D]
        in_maps.append({"pk": pk, "wk": wk})
    return in_maps


def _run(selection_score, expert_indices, all_weight, trace=False):
    from concourse.bass_utils import run_bass_kernel_spmd

    in_maps = _prep_inputs(selection_score, expert_indices, all_weight)
    if "nc" not in _cache:
        _cache["nc"] = _build_program()
    nc = _cache["nc"]

    r = run_bass_kernel_spmd(nc, in_maps, list(range(N_CORES)), trace=trace)
    full = np.empty((BS, NF), dtype=np.float32)
    for core in range(N_CORES):
        rg, cg = divmod(core, CG)
        full[rg * ROWS : (rg + 1) * ROWS, cg * COLS : (cg + 1) * COLS] = r.results[
            core
        ]["out"]
    return full.reshape(BS, PL, D), r


def kernel(selection_score, expert_indices, all_weight) -> np.ndarray:
    full, _ = _run(selection_score, expert_indices, all_weight, trace=False)
    return full


# revision 105
# speedup vs baseline: 1.0482x; 1.0482x over previous
"""MoE routing mixture kernel for Trainium2 (8 NeuronCores, SPMD).

Math: out[b] = sum_k selection_score[b, idx[b,k]] * all_weight[idx[b,k]]
Rewritten as a dense matmul: out = C @ W_flat, where
  C[b,e]    = selection_score[b,e] * |{k : idx[b,k]==e}|      ([2048, 64])
  W_flat    = all_weight.reshape(64, 16384)

Sharding: 8 cores = 2 row-groups x 4 col-groups. Each core produces a
[1024, 4096] tile of the [2048, 16384] output. The big store is fp16
(the problem is DMA-roofline bound: fp32 out would be 16.8 MB/core,
fp16 is 8.4 MB; W slice per core is [64, 4096] fp16 = 0.5 MB).

Engine constraints that shape the design: DMA cannot touch PSUM, and
GPSIMD (Pool) cannot access PSUM either, so every matmul result must
pass through an ACT or DVE PSUM->SBUF copy — those copies are the
second-tightest resource after the DMA. Pool therefore handles the
SBUF-only one-hot work.

Per-core pipeline (raw Bass):
  SP  : loads in dependency order (mini: idx bits; rest: scoresT +
        W slice 0; W slices 1-3; W slices 4-7), then 33 output stores.
  Pool: generates iota + transpose identity on-chip at t~0, then per
        128-row chunk r>=1: 8x tensor_scalar is_equal + add tree
        -> cnt[r] (all SBUF; GPSIMD cannot touch PSUM).
  DVE : chunk 0's eq/reduce (fill path), then per chunk: ct[r] =
        ctp * scoresT (PSUM->SBUF move of C^T fused with the score
        multiply) + 3-4 of the 8 PSUM cast-copy slices.
  PE  : ~28 dummy transposes from t~0 (clock-ramp warm-up), then per
        chunk: transpose cnt[r] -> ctp (fp16 PSUM, hoisted between the
        previous chunk's matmuls); 8 fp16 matmuls [64,128]^T @ [64,512]
        -> fp32 PSUM, rotating over a 7-bank ring (bank = matmul % 7).
  ACT : 4-5 of the 8 cast-copy slices per chunk.

Output assembled on host: fp16 tiles -> fp32 [2048, 32, 512].
"""

import sys
from contextlib import ExitStack

import numpy as np

sys.path.insert(0, "/opt/trn_rl_repo")

BS, E, TOPK, PL, D = 2048, 64, 8, 32, 512
NF = PL * D  # 16384 flattened prompt*dim
N_CORES = 8
RG, CG = 2, 4  # row groups x col groups
ROWS = BS // RG  # 1024 rows per core
COLS = NF // CG  # 4096 cols per core
RCH = ROWS // 128  # 8 row chunks
NSL = COLS // D  # 8 matmul slices of 512 cols per chunk
NPS = 7  # psum ring banks

# packed small-input layout (fp16 cols), loaded as two DMAs:
#   mini [0 : 128)    = [128 idx-as-f32-bits]   (unblocks the eq chains)
#   rest [128 : 1152) = [512 scoT2 | 512 W slice 0 (partitions 0:64)]
# (iota and the transpose identity are generated on-chip by Pool)
PK_IDX = 0
PK_SCO = 2 * RCH * TOPK
PK_W0 = PK_SCO + 512
PKW = PK_W0 + D
PK_MINI = PK_SCO  # boundary between the two loads


def _group_units(g):
    """Copy units (tuples of slice indices) for group g, split by engine.

    The PSUM bank of matmul m is m % 7, so bank adjacency inside a group
    rotates with g; pairs are chosen so both slices sit in adjacent banks
    (single contiguous copy). ACT gets 5 slices (2 pairs + slice 7), DVE
    3 (1 pair + 1 single). The DVE single goes first when its ring
    deadline is tight (low slice index).
    """
    if g == 0:
        # fill path: all singles early; DVE does slice 0 inline right
        # after mul(0) while otherwise idle, so the first store (a [512]
        # store of slice 0) can issue as soon as possible
        return [(1,), (4, 5), (7,)], [(0,), (2, 3), (6,)]
    G = g % 7
    if G == 2:
        pairs, singles = [(0, 1), (2, 3), (5, 6)], [4, 7]
    elif G == 4:
        pairs, singles = [(0, 1), (3, 4), (5, 6)], [2, 7]
    elif G == 6:
        pairs, singles = [(1, 2), (3, 4), (5, 6)], [0, 7]
    else:
        pairs, singles = [(0, 1), (2, 3), (4, 5)], [6, 7]
    act = [pairs[0], pairs[2]]
    if singles[0] < pairs[1][0]:
        dve = [(singles[0],), pairs[1]]
    else:
        dve = [pairs[1], (singles[0],)]
    # slice 7 alternates between ACT and DVE to balance the two PSUM-copy
    # engines (~2.4us/chunk each; the 2.91us DMA store rate then paces)
    if g % 2 == 0:
        act.append((singles[1],))
    else:
        dve.append((singles[1],))
    return act, dve


def _copy_tables():
    """slice index (0..63) -> (engine, count) + per-group emission lists."""
    table = {}
    counts = {"A": 0, "D": 0}
    act_em, dve_em = [], []
    for g in range(RCH):
        act, dve = _group_units(g)
        act_em.append(act)
        dve_em.append(dve)
        for unit in act:
            counts["A"] += 1
            for s in unit:
                table[NSL * g + s] = ("A", counts["A"])
        for unit in dve:
            counts["D"] += 1
            for s in unit:
                table[NSL * g + s] = ("D", counts["D"])
    return table, act_em, dve_em


_SLICE_SEM, _ACT_EM, _DVE_EM = _copy_tables()

_cache: dict = {}


def _done_waits(*slices):
    need = {}
    for s in slices:
        sem, n = _SLICE_SEM[s]
        need[sem] = max(need.get(sem, 0), n)
    return sorted(need.items())


def _build_program():
    import concourse.bass as bass
    import concourse.mybir as mybir

    f16 = mybir.dt.float16
    f32 = mybir.dt.float32
    eq_op = mybir.AluOpType.is_equal
    add_op = mybir.AluOpType.add
    nc = bass.Bass()

    pk_d = nc.declare_dram_parameter("pk", [128, PKW], f16, isOutput=False)
    w_d = nc.declare_dram_parameter("wk", [E, COLS], f16, isOutput=False)
    out_d = nc.declare_dram_parameter("out", [ROWS, COLS], f16, isOutput=True)

    ctx = ExitStack()
    with ctx:
        sb = lambda tag, shape, dt=f16: ctx.enter_context(  # noqa: E731
            nc.sbuf_tensor(tag, shape, dt)
        )
        pk_t = sb("pk_t", [128, PKW])
        w_t = sb("w_t", [E, COLS])
        wu_t = sb("wu_t", [128, 128])  # never written: warm-up operand
        eq = [sb(f"eq{r}", [128, TOPK * E]) for r in range(RCH)]
        eq0 = eq[0]
        tr1 = sb("tr1", [128, 4 * E])
        tr2 = sb("tr2", [128, 2 * E])
        cnt = [sb(f"cnt{r}", [128, E]) for r in range(RCH)]
        ct = [sb(f"ct{r}", [E, 128]) for r in range(RCH)]
        stg = [sb(f"stg{r}", [128, COLS]) for r in range(RCH)]
        # on-chip constants (Pool generates these at t~0)
        i16 = mybir.dt.int16
        it_pm = sb("it_pm", [128, 128], i16)
        it_e = sb("it_e", [128, E], i16)
        iota_t = sb("iota_t", [128, E])
        ident_t = sb("ident_t", [128, 128])

        iota_ap = iota_t[:]
        ident_ap = ident_t[:]

        def idx_scalar(r, k):
            c = PK_IDX + 2 * (r * TOPK + k)
            return pk_t[:, c : c + 2].bitcast(f32)

        def scoT_ap(r):
            # scoT2[p, c]: p<64 -> scores.T[p, c]; p>=64 -> scores.T[p-64, 512+c]
            pbase = (r // 4) * E
            cbase = PK_SCO + (r % 4) * 128
            return pk_t[pbase : pbase + E, cbase : cbase + 128]

        ctp = ctx.enter_context(nc.psum_tensor("ctp", [E, 128], f16))
        pmall = ctx.enter_context(nc.psum_tensor("pmall", [128, NPS * D], f32))

        def pm_ap(bank, nbanks=1):
            return pmall[:, bank * D : (bank + nbanks) * D]

        s_in = ctx.enter_context(nc.semaphore("s_in"))
        s_mini = ctx.enter_context(nc.semaphore("s_mini"))
        s_w = ctx.enter_context(nc.semaphore("s_w"))
        s_const = ctx.enter_context(nc.semaphore("s_const"))
        s_eq0 = ctx.enter_context(nc.semaphore("s_eq0"))
        s_cnt = ctx.enter_context(nc.semaphore("s_cnt"))
        s_tp = ctx.enter_context(nc.semaphore("s_tp"))
        s_c = ctx.enter_context(nc.semaphore("s_c"))
        s_mm = ctx.enter_context(nc.semaphore("s_mm"))
        s_cp = {
            "A": ctx.enter_context(nc.semaphore("s_cpa")),
            "D": ctx.enter_context(nc.semaphore("s_cpd")),
        }
        s_out = ctx.enter_context(nc.semaphore("s_out"))

        def emit_unit_copy(engobj, copy_fn, sem, g, unit):
            """PSUM->SBUF fp32->fp16 cast copy of `unit` (adjacent banks).
            The s_mm wait rides on the copy instruction itself (one wait +
            one update per descriptor), saving a standalone-wait SEQ slot."""
            j0, jn = unit[0], unit[-1]
            m_last = NSL * g + jn
            bank = (NSL * g + j0) % NPS
            ins = copy_fn(stg[g][:, j0 * D : (jn + 1) * D], pm_ap(bank, len(unit)))
            ins.wait_op(s_mm, m_last + 1, "sem-ge")
            ins.then_inc(sem, 1)

        block = ctx.enter_context(nc.Block())

        @block.sync
        def _(sp):
            sp.dma_start(out=pk_t[:, :PK_MINI], in_=pk_d[:, :PK_MINI]).then_inc(
                s_mini, 16
            )
            sp.dma_start(out=pk_t[:, PK_MINI:], in_=pk_d[:, PK_MINI:]).then_inc(
                s_in, 16
            )
            # W slice 0 rides in the rest load; slices 1-3 and 4-7 stream in
            sp.dma_start(out=w_t[:, D : COLS // 2], in_=w_d[:, D : COLS // 2]).then_inc(
                s_w, 16
            )
            sp.dma_start(out=w_t[:, COLS // 2 :], in_=w_d[:, COLS // 2 :]).then_inc(
                s_w, 16
            )
            # group 0 ships as 2x[512] + 3x[1024]; groups 1..7 as 4x[1024]
            stores = [(0, (0, 0)), (0, (1, 1)), (0, (2, 3)), (0, (4, 5)), (0, (6, 7))]
            stores += [
                (r, (2 * q, 2 * q + 1)) for r in range(1, RCH) for q in range(4)
            ]
            for r, (j0, j1) in stores:
                waits = _done_waits(NSL * r + j0, NSL * r + j1)
                # attach one wait to the DMA itself: its 565ns SEQ setup then
                # runs before the wait instead of after, so the transfer
                # launches sooner once the data lands (extras go standalone)
                for eng, n in waits[:-1]:
                    sp.wait_ge(s_cp[eng], n)
                rows = slice(r * 128, (r + 1) * 128)
                cols = slice(j0 * D, (j1 + 1) * D)
                ins = sp.dma_start(out=out_d[rows, cols], in_=stg[r][:, cols])
                eng, n = waits[-1]
                ins.wait_op(s_cp[eng], n, "sem-ge")
                ins.then_inc(s_out, 16)
            sp.wait_ge(s_out, 16 * len(stores))

        @block.gpsimd
        def _(gp):
            # on-chip constants, ready by t~1.5us (before any load completes):
            # iota row 0..63 and the 128x128 transpose identity
            gp.iota(it_e[:], [[1, E]], base=0, channel_multiplier=0)
            gp.tensor_copy(iota_t[:], it_e[:]).then_inc(s_const, 1)
            # it_pm[p, j] = j - p; identity = (it_pm == 0)
            gp.iota(it_pm[:], [[1, 128]], base=0, channel_multiplier=-1)
            gp.tensor_scalar(
                ident_t[:], it_pm[:], 0.0, None, eq_op
            ).then_inc(s_const, 1)
            # one-hot counts for chunks 1..7 (chunk 0 runs on DVE for a
            # faster pipeline fill); all operands SBUF-only. NOTE: Pool's
            # legal op set on V3 is narrow — tensor_tensor with broadcast
            # APs and scalar_tensor_tensor both fail the engine check, so
            # this stays 8x tensor_scalar + an add tree.
            for r in range(1, RCH):
                for k in range(TOPK):
                    ins = gp.tensor_scalar(
                        eq[r][:, k * E : (k + 1) * E],
                        iota_ap,
                        idx_scalar(r, k),
                        None,
                        eq_op,
                    )
                    if r == 1 and k == 0:
                        ins.wait_op(s_mini, 16, "sem-ge")
                gp.drain()
                gp.tensor_tensor(
                    tr1[:], eq[r][:, : 4 * E], eq[r][:, 4 * E :], add_op
                )
                gp.drain()
                gp.tensor_tensor(tr2[:], tr1[:, : 2 * E], tr1[:, 2 * E :], add_op)
                gp.drain()
                gp.tensor_tensor(cnt[r][:], tr2[:, :E], tr2[:, E:], add_op).then_inc(
                    s_cnt, 1
                )

        @block.vector
        def _(v):
            v.wait_ge(s_const, 1)  # iota generated by Pool
            # chunk 0 count path on DVE (idle during fill). NOTE: summing the
            # one-hots via accumulating fp16 transposes in PSUM works in the
            # simulator but produces wrong results on hardware — keep this
            # eq + tensor_reduce + single transpose formulation. The mini-load
            # wait rides on the first eq.
            for k in range(TOPK):
                ins = v.tensor_scalar(
                    eq0[:, k * E : (k + 1) * E],
                    iota_ap,
                    idx_scalar(0, k),
                    None,
                    eq_op,
                )
                if k == 0:
                    ins.wait_op(s_mini, 16, "sem-ge")
            v.drain()
            # counts are small integers (<= 8): exact in fp16
            with nc.allow_low_precision(reason="counts <= 8 are exact in fp16"):
                v.tensor_reduce(
                    cnt[0][:],
                    eq0[:].rearrange("p (k e) -> p e k", k=TOPK),
                    mybir.AxisListType.X,
                    add_op,
                ).then_inc(s_eq0, 1)
            v.wait_ge(s_in, 16)  # scoT arrives with the rest load
            for r in range(RCH):
                if r == 1:
                    # chunk 0's pair copy before mul(1): its matmuls are done
                    # while mul(1) still waits on the hoisted transpose
                    emit_unit_copy(v, v.tensor_copy, s_cp["D"], 0, _DVE_EM[0][1])
                ins = v.tensor_mul(ct[r][:], ctp[:], scoT_ap(r))
                ins.wait_op(s_tp, r + 1, "sem-ge")
                ins.then_inc(s_c, 1)
                if r == 0:
                    # slice 0 copy inline: DVE idle during fill
                    emit_unit_copy(v, v.tensor_copy, s_cp["D"], 0, _DVE_EM[0][0])
                else:
                    units = _DVE_EM[r - 1]
                    for unit in units[2:] if r == 1 else units:
                        emit_unit_copy(v, v.tensor_copy, s_cp["D"], r - 1, unit)
            for unit in _DVE_EM[RCH - 1]:
                emit_unit_copy(v, v.tensor_copy, s_cp["D"], RCH - 1, unit)

        @block.tensor
        def _(t):
            def tp(r):
                # ctp's previous reader (mul of r-1) is already done when
                # this runs: G(r-1) started, which required s_c >= r
                ins = t.transpose(ctp[:], cnt[r][:], ident_ap)
                ins.wait_op(s_cnt, r, "sem-ge")
                ins.then_inc(s_tp, 1)

            # pstate warm-up: the PE clock ramps to full only after ~3us of
            # near-continuous execution, and the ramp clock resets on long
            # idle gaps. Keep PE busy with dummy transposes (operand values
            # irrelevant; ctp is overwritten below) from t~0 until the
            # first real matmul so the whole fill phase runs at full clock.
            for _ in range(28):
                t.transpose(ctp[:], wu_t[:, 0:E], wu_t[:])
            t.wait_ge(s_const, 2)  # ident generated by Pool
            ins = t.transpose(ctp[:], cnt[0][:], ident_ap)
            ins.wait_op(s_eq0, 1, "sem-ge")  # cnt[0] from DVE
            ins.then_inc(s_tp, 1)
            for r in range(RCH):
                for j in range(NSL):
                    if r == 0 and j == 1:
                        t.wait_ge(s_w, 16)  # W slices 1-3
                    if r == 0 and j == 4:
                        t.wait_ge(s_w, 32)  # W slices 4-7
                    # hoist next chunk's transpose between matmuls, at the
                    # slot matching when Pool's count arrives (Pool lags
                    # early on and catches up by ~chunk 4) so PE never
                    # stalls mid-group before its ring-critical matmuls
                    if j == {0: 5, 1: 5, 2: 5, 3: 4}.get(r, 2) and r + 1 < RCH:
                        tp(r + 1)
                    m = r * NSL + j
                    ring = _done_waits(m - NPS) if m >= NPS else []
                    for eng, n in ring[:-1]:
                        t.wait_ge(s_cp[eng], n)
                    # W slice 0 lives in the pk tensor (rides the rest load)
                    rhs = (
                        pk_t[:E, PK_W0:] if j == 0 else w_t[:, j * D : (j + 1) * D]
                    )
                    if j == 0 and ring:
                        # first matmul carries the ct-ready wait; its ring
                        # wait (bank of m-7) then goes standalone
                        eng, n = ring.pop()
                        t.wait_ge(s_cp[eng], n)
                    ins = t.matmul(
                        pm_ap(m % NPS),
                        ct[r][:],
                        rhs,
                        start=True,
                        stop=True,
                    )
                    if j == 0:
                        ins.wait_op(s_c, r + 1, "sem-ge")  # ct[r] ready
                    elif ring:
                        # bank m%7 was last written by matmul m-7
                        eng, n = ring[-1]
                        ins.wait_op(s_cp[eng], n, "sem-ge")
                    ins.then_inc(s_mm, 1)

        @block.scalar
        def _(a):
            for r in range(RCH):
                for unit in _ACT_EM[r]:
                    emit_unit_copy(a, a.copy, s_cp["A"], r, unit)

    return nc


def _prep_inputs(selection_score, expert_indices, all_weight):
    scores = np.asarray(selection_score, dtype=np.float32)
    idx = np.asarray(expert_indices)
    w = np.asarray(all_weight, dtype=np.float32).reshape(E, NF).astype(np.float16)

    in_maps = []
    for core in range(N_CORES):
        rg, cg = divmod(core, CG)
        rsl = slice(rg * ROWS, (rg + 1) * ROWS)
        scoT = scores[rsl].T.astype(np.float16)  # [64, 1024]
        idxp = np.ascontiguousarray(
            idx[rsl]
            .astype(np.float32)
            .reshape(RCH, 128, TOPK)
            .transpose(1, 0, 2)
            .reshape(128, RCH * TOPK)
        )
        wk = np.ascontiguousarray(w[:, cg * COLS : (cg + 1) * COLS])
        pk = np.zeros((128, PKW), dtype=np.float16)
        pk[:, PK_IDX:PK_SCO] = idxp.view(np.float16)
        pk[:E, PK_SCO:PK_W0] = scoT[:, :512]
        pk[E:, PK_SCO:PK_W0] = scoT[:, 512:]
        pk[:E, PK_W0:] = wk[:, :D]
        in_maps.append({"pk": pk, "wk": wk})
    return in_maps


def _run(selection_score, expert_indices, all_weight, trace=False):
    from concourse.bass_utils import run_bass_kernel_spmd

    in_maps = _prep_inputs(selection_score, expert_indices, all_weight)
    if "nc" not in _cache:
        _cache["nc"] = _build_program()
    nc = _cache["nc"]

    r = run_bass_kernel_spmd(nc, in_maps, list(range(N_CORES)), trace=trace)
    full = np.empty((BS, NF), dtype=np.float32)
    for core in range(N_CORES):
        rg, cg = divmod(core, CG)
        full[rg * ROWS : (rg + 1) * ROWS, cg * COLS : (cg + 1) * COLS] = r.results[
            core
        ]["out"]
    return full.reshape(BS, PL, D), r


def kernel(selection_score, expert_indices, all_weight) -> np.ndarray:
    full, _ = _run(selection_score, expert_indices, all_weight, trace=False)
    return full
